# revision 55
# baseline (speedup 1.0000x reference)
"""Trainium2 Bass kernel for nn_DropScaledDotProductAttention.

Reference computation (B=2, N=16 heads, LQ=512, DK=DV=64, DD=128, h2=256):
    f[b,i,j]   = concat(d0[b,i], d1[b,j])                      [b,lq,lq,256]
    h          = relu(f @ W1 + b1)                             [b,lq,lq,256]
    logits     = h @ W2 + b2                                   [b,lq,lq,2]
    decisions  = argmax(logits) == 1                           [b,1,lq,lq]
    attn       = softmax(where(dec, (q/8) @ k^T, -1e9))        [b,n,lq,lq]
    output     = attn @ v                                      [b,n,lq,dv]
    returns (output, attn, decisions)

Key algebraic restructuring: since f is a concat, f @ W1 splits into
    A[b,i,:] = d0[b,i] @ W1[:128,:]      (per-query part)
    Bt[b,j,:] = d1[b,j] @ W1[128:,:] + b1 (per-key part)
so h[b,i,j,:] = relu(A[b,i,:] + Bt[b,j,:]).  The decision only needs the
sign of logits[...,1]-logits[...,0] = relu(A_i+B_j) . (W2[:,1]-W2[:,0]) +
(b2[1]-b2[0]).  This removes the 68-GFLOP pairwise GEMM entirely: the
remaining per-pair work is one relu pass (ACT/DVE) and one 256-length dot
(PE, streaming X columns against a resident weight vector).

Sharding: 8 cores = 2 batches x 4 query-blocks of 128.  Each core computes
its [128, 512] decision block once and reuses it across all 16 heads.
"""

import os
import sys

import numpy as np

sys.path.insert(0, "/opt/trn_rl_repo")

import concourse.bass as bass
import concourse.mybir as mybir
from concourse import tile
from concourse.vector_clock import ScopedClock, VectorClock
from concourse.bass_utils import run_bass_kernel_spmd

F32 = mybir.dt.float32
U8 = mybir.dt.uint8
AF = mybir.ActivationFunctionType
ALU = mybir.AluOpType

B, N, LQ, DK, DV, DD = 2, 16, 512, 64, 64, 128
H2 = 2 * DD
IBLK = 128            # query rows per core
NCORES = 8

NEG = -1.0e9


def R(ap):
    """View an fp32 AP as float32r: PE streams fp32r at full rate for
    moving free-dim >= 256 (plain fp32 matmul costs 4 cycles/row)."""
    return ap.bitcast(mybir.dt.float32r)

# ---------------------------------------------------------------------------
# This container's walrus accepts at most ONE semaphore wait per instruction
# ("Too many sync wait commands"), while Tile freely attaches several.  Split:
# keep the last wait on the instruction and hoist the rest onto EventSemaphore
# instructions inserted just before it on the same engine stream.
# ---------------------------------------------------------------------------


def _split_multiwaits(nc: "bass.Bass") -> int:
    n_split = 0
    for f in nc.m.functions:
        for bb in f.blocks:
            insts = bb.instructions
            pos = 0
            while pos < len(insts):
                ins = insts[pos]
                si = ins.sync_info
                if si is not None and len(si.on_wait) > 1:
                    waits = list(si.on_wait)
                    for wi, w in enumerate(waits[:-1]):
                        es = mybir.InstEventSemaphore(
                            name=f"{ins.name}-wsplit{wi}", ins=[], outs=[]
                        )
                        es.engine = ins.engine
                        es.sync_info = mybir.SyncInfo(on_wait=[w], on_update=[])
                        insts.insert(pos, es)
                        pos += 1
                    ins.sync_info = mybir.SyncInfo(
                        on_wait=[waits[-1]], on_update=list(si.on_update)
                    )
                    n_split += 1
                pos += 1
    return n_split


# ---------------------------------------------------------------------------
# Device program (SPMD: identical program, per-core shards)
# ---------------------------------------------------------------------------


def _build_nc(delta: float) -> bass.Bass:
    nc = bass.Bass("TRN2", debug=False, num_devices=NCORES)

    # inputs (per-core shapes)
    qT = nc.dram_tensor("qT", [DK, N * IBLK], F32, kind="ExternalInput")
    kT = nc.dram_tensor("kT", [DK, N * LQ], F32, kind="ExternalInput")
    vS = nc.dram_tensor("vS", [128, N * (LQ // 128) * DV], F32, kind="ExternalInput")
    d0T = nc.dram_tensor("d0T", [DD, IBLK], F32, kind="ExternalInput")
    d1T = nc.dram_tensor("d1T", [DD, LQ], F32, kind="ExternalInput")
    W1d = nc.dram_tensor("W1d", [H2, H2], F32, kind="ExternalInput")
    # wexp[:, (h*32+r)*32 + c] = w_half_h[g] if c == r else 0 — places row
    # i's dot at PSUM partition 32*(i//32) + (i%32) via a one-hot weight col
    wexpd = nc.dram_tensor("wexpd", [DD, 2048], F32, kind="ExternalInput")
    b1d = nc.dram_tensor("b1d", [DD, 2], F32, kind="ExternalInput")

    # outputs
    attn_out = nc.dram_tensor("attn", [N, IBLK, LQ], F32, kind="ExternalOutput")
    outp_out = nc.dram_tensor("outp", [N, IBLK, DV], F32, kind="ExternalOutput")
    dec_out = nc.dram_tensor("dec", [IBLK, LQ], U8, kind="ExternalOutput")

    ident_d = nc.inline_tensor(np.eye(128, dtype=np.float32), name="ident128")

    NJB = LQ // 128  # 4 key blocks

    with tile.TileContext(nc) as tc:
        with (
            tc.tile_pool(name="const", bufs=1) as cpool,
            tc.tile_pool(name="xp", bufs=12) as xpool,
            tc.tile_pool(name="wk", bufs=2) as wk,
            tc.tile_pool(name="wks", bufs=3) as wks,
        ):
            # ---- load constants / inputs ------------------------------------
            W1t = cpool.tile([DD, H2], F32)       # rows 0:128 of W1 (d0 part)
            W1b = cpool.tile([DD, H2], F32)       # rows 128:256 of W1 (d1 part)
            d0T_sb = cpool.tile([DD, IBLK], F32)
            d1T_sb = cpool.tile([DD, LQ], F32)
            wexp_sb = cpool.tile([DD, 2048], F32)
            b1_sb = cpool.tile([DD, 2], F32)
            ident = cpool.tile([128, 128], F32)
            qT_sb = cpool.tile([DK, N * IBLK], F32)
            kT_sb = cpool.tile([DK, N * LQ], F32)
            v_sb = cpool.tile([128, N * NJB * DV], F32)

            nc.sync.dma_start(R(d1T_sb[:]), R(d1T[:]))
            nc.sync.dma_start(R(W1b[:]), R(W1d[DD:H2, :]))
            nc.sync.dma_start(b1_sb[:], b1d[:])
            nc.sync.dma_start(R(d0T_sb[:]), R(d0T[:]))
            nc.sync.dma_start(R(W1t[:]), R(W1d[0:DD, :]))
            nc.sync.dma_start(R(wexp_sb[:]), R(wexpd[:]))
            nc.sync.dma_start(ident[:], ident_d[:])
            nc.sync.dma_start(R(qT_sb[:]), R(qT[:]))
            nc.sync.dma_start(R(kT_sb[:]), R(kT[:]))
            nc.sync.dma_start(v_sb[:], vS[:])

            # ---- A/B feature GEMMs -----------------------------------------
            # A_T[g, i] = sum_d W1_top[d, g] * d0T[d, i]   (two g-halves)
            A_sb = [cpool.tile([DD, IBLK], F32, name=f"Asb{h}", tag=f"A{h}")
                    for h in range(2)]
            B_sb = [cpool.tile([DD, LQ], F32, name=f"Bsb{h}", tag=f"B{h}")
                    for h in range(2)]
            with tc.tile_pool(name="ps_ab", bufs=2, space="PSUM") as psab:
                for h in range(2):
                    pa = psab.tile([DD, IBLK], F32, tag="pa")
                    nc.tensor.matmul(
                        pa[:], R(W1t[:, h * DD:(h + 1) * DD]), R(d0T_sb[:]),
                        start=True, stop=True,
                    )
                    nc.scalar.copy(A_sb[h][:], pa[:])
                for h in range(2):
                    pb = psab.tile([DD, LQ], F32, tag="pb")
                    nc.tensor.matmul(
                        pb[:], R(W1b[:, h * DD:(h + 1) * DD]), R(d1T_sb[:]),
                        start=True, stop=True,
                    )
                    # B += b1 (per-partition scalar) while evacuating PSUM
                    nc.vector.tensor_scalar_add(
                        B_sb[h][:], pb[:], b1_sb[:, h:h + 1]
                    )

            # ---- decision block: D[i,j] = relu(A_i+B_j) . w + delta ---------
            # PE matmul rows land at partition 32*col_grp + r, where r is the
            # one-hot column of the expanded weight tile wexp.  Row i's two
            # half-dots accumulate at partition i of a single PSUM bank; the
            # other 31 rows of each matmul add exact zeros.
            m_sb = cpool.tile([IBLK, LQ], F32)
            with tc.tile_pool(name="ep", bufs=N) as epool:
              with (
                tc.tile_pool(name="ps_dec", bufs=1, space="PSUM") as psd,
                tc.tile_pool(name="ps_s", bufs=4, space="PSUM") as pss,
              ):
                e_tiles = []
                ps_dec = [psd.tile([32, LQ], F32, name=f"psd{c}", tag=f"d{c}")
                          for c in range(4)]
                for i in range(IBLK):
                    # interleave decision-independent attention work (QK +
                    # exp) into the decision phase: one head per 8 rows
                    if i % 8 == 7:
                        hh = i // 8
                        s_ps = pss.tile([IBLK, LQ], F32, tag="s")
                        nc.tensor.matmul(
                            s_ps[:],
                            R(qT_sb[:, hh * IBLK:(hh + 1) * IBLK]),
                            R(kT_sb[:, hh * LQ:(hh + 1) * LQ]),
                            start=True, stop=True,
                        )
                        e_sb = epool.tile([IBLK, LQ], F32, tag="e")
                        nc.scalar.activation(e_sb[:], s_ps[:], AF.Exp)
                        e_tiles.append(e_sb)
                    cg, r = i // 32, i % 32
                    xs = []
                    for h in range(2):
                        x = xpool.tile([DD, LQ], F32, tag="x")
                        idx = i * 2 + h
                        sel = idx % 8
                        if sel == 0 or (sel == 7 and (idx // 8) % 2 == 0):
                            # ACT path: relu(1.0*B + A[:,i])
                            nc.scalar.activation(
                                R(x[:]), B_sb[h][:], AF.Relu,
                                bias=A_sb[h][:, i:i + 1], scale=1.0,
                            )
                        elif sel == 4 or (sel == 7 and (idx // 8) % 2 == 1):
                            # GPSIMD path (otherwise-idle engine)
                            nc.gpsimd.tensor_scalar(
                                R(x[:]), B_sb[h][:], A_sb[h][:, i:i + 1], 0.0,
                                ALU.add, ALU.max,
                            )
                        else:
                            # DVE path: max(B + A[:,i], 0)
                            nc.vector.tensor_scalar(
                                R(x[:]), B_sb[h][:], A_sb[h][:, i:i + 1], 0.0,
                                ALU.add, ALU.max,
                            )
                        xs.append(x)
                    out_ap = ps_dec[cg][0:32, :]
                    for h in range(2):
                        nc.tensor.matmul(
                            out_ap,
                            R(wexp_sb[:, (h * 32 + r) * 32:(h * 32 + r) * 32 + 32]),
                            R(xs[h][:]),
                            start=(r == 0 and h == 0),
                            stop=(r == 31 and h == 1),
                        )

                # m = (D + delta) > 0  as 1.0/0.0
                for c in range(4):
                    nc.vector.tensor_scalar(
                        m_sb[c * 32:(c + 1) * 32, :], ps_dec[c][0:32, :],
                        float(delta), 0.0, ALU.add, ALU.is_gt
                    )
                # decisions out as u8
                dec_u8 = cpool.tile([IBLK, LQ], U8)
                nc.gpsimd.tensor_copy(dec_u8[:], m_sb[:])
                nc.sync.dma_start(dec_out[:], dec_u8[:])

              with (
                tc.tile_pool(name="ps_t", bufs=2, space="PSUM") as pst,
                tc.tile_pool(name="ps_o", bufs=1, space="PSUM") as pso,
              ):
                # ---- attention tail: mask E = exp(S) by m and normalize ----
                # P = E*m is exact: exp(-1e9) == 0 in f32, so this matches
                # where(dec, S, -1e9); the zc-blend reproduces the exact
                # uniform-1/512 fully-masked rows.  Software-pipelined with
                # per-stage head lag so every engine stream interleaves
                # different heads' stages without cross-stage stalls.
                rs_t, zc_t, ri_t, a_t, tp_t, ts_t, op_t = ({} for _ in range(7))

                def st_mask(h):
                    nc.vector.tensor_tensor(
                        e_tiles[h][:], e_tiles[h][:], m_sb[:], ALU.mult)

                def st_rowsum(h):
                    p2 = wk.tile([IBLK, LQ], F32, name=f"p2_{h}", tag=f"p2{h % 2}")
                    rs_t[h] = wks.tile([IBLK, 1], F32, name=f"rs{h}", tag=f"rs{h % 4}")
                    nc.scalar.activation(
                        p2[:], e_tiles[h][:], AF.Copy, accum_out=rs_t[h][:])

                def st_small(h):
                    zc_t[h] = wks.tile([IBLK, 1], F32, name=f"zc{h}", tag=f"zc{h % 4}")
                    nc.gpsimd.tensor_scalar(
                        zc_t[h][:], rs_t[h][:], 0.0, 1.0 / float(LQ),
                        ALU.is_equal, ALU.mult)
                    rs2 = wks.tile([IBLK, 1], F32, name=f"rs2_{h}", tag=f"rs2{h % 4}")
                    nc.gpsimd.tensor_scalar(
                        rs2[:], rs_t[h][:], zc_t[h][:, 0:1], None, ALU.add)
                    ri_t[h] = wks.tile([IBLK, 1], F32, name=f"ri{h}", tag=f"ri{h % 4}")
                    nc.vector.reciprocal(ri_t[h][:], rs2[:])

                def st_norm(h):
                    a_t[h] = wk.tile([IBLK, LQ], F32, name=f"a{h}", tag=f"a{h % 4}")
                    eng = nc.vector if h % 3 == 0 else nc.gpsimd
                    eng.tensor_scalar(
                        a_t[h][:], e_tiles[h][:], ri_t[h][:, 0:1], zc_t[h][:, 0:1],
                        ALU.mult, ALU.add)
                    nc.sync.dma_start(attn_out[h], a_t[h][:])

                def st_av(h):
                    tp_t[h] = pst.tile([128, LQ], F32, name=f"tp{h}", tag=f"t{h % 2}")
                    for jb in range(NJB):
                        nc.tensor.transpose(
                            tp_t[h][:, jb * 128:(jb + 1) * 128],
                            a_t[h][:, jb * 128:(jb + 1) * 128],
                            ident[:],
                        )
                    ts_t[h] = wk.tile([128, LQ], F32, name=f"tsb{h}", tag=f"tsb{h % 4}")
                    nc.scalar.copy(ts_t[h][:, 0:256], tp_t[h][:, 0:256])
                    nc.vector.tensor_copy(ts_t[h][:, 256:512], tp_t[h][:, 256:512])
                    op_t[h] = pso.tile([IBLK, DV], F32, name=f"op{h}", tag=f"o{h % 3}")
                    for jb in range(NJB):
                        nc.tensor.matmul(
                            op_t[h][:],
                            ts_t[h][:, jb * 128:(jb + 1) * 128],
                            v_sb[:, (h * NJB + jb) * DV:(h * NJB + jb + 1) * DV],
                            start=(jb == 0), stop=(jb == NJB - 1),
                        )
                    o_sb = wks.tile([IBLK, DV], F32, name=f"ob{h}", tag=f"ob{h % 4}")
                    nc.vector.tensor_copy(o_sb[:], op_t[h][:])
                    nc.sync.dma_start(outp_out[h], o_sb[:])

                stages = [st_mask, st_rowsum, st_small, st_norm, st_av]
                for slot in range(N + len(stages) - 1):
                    for k, st in enumerate(stages):
                        h = slot - k
                        if 0 <= h < N:
                            st(h)

    _split_multiwaits(nc)
    return nc


_NC_CACHE: dict = {}
LAST_RESULTS = None


def kernel(q, k, v, d0, d1, W1, b1, W2, b2):
    global LAST_RESULTS
    q = np.asarray(q, np.float32)
    k = np.asarray(k, np.float32)
    v = np.asarray(v, np.float32)
    d0 = np.asarray(d0, np.float32)
    d1 = np.asarray(d1, np.float32)
    W1 = np.asarray(W1, np.float32)
    b1 = np.asarray(b1, np.float32)
    W2 = np.asarray(W2, np.float32)
    b2 = np.asarray(b2, np.float32)

    delta = float(np.float32(b2[1]) - np.float32(b2[0]))
    key = delta
    nc = _NC_CACHE.get(key)
    if nc is None:
        nc = _build_nc(delta)
        _NC_CACHE[key] = nc

    w = (W2[:, 1] - W2[:, 0]).astype(np.float32)
    # wexp[g, h, r, c] = w_half_h[g] * (c == r)
    wexp = np.zeros((DD, 2, 32, 32), np.float32)
    for r in range(32):
        wexp[:, 0, r, r] = w[:DD]
        wexp[:, 1, r, r] = w[DD:]
    wexp = np.ascontiguousarray(wexp.reshape(DD, 2048))
    b1c = np.stack([b1[:DD], b1[DD:]], axis=1).astype(np.float32)

    in_maps = []
    for c in range(NCORES):
        b = c // 4
        i0 = (c % 4) * IBLK
        qs = (q[b, :, i0:i0 + IBLK, :] * 0.125).astype(np.float32)
        in_maps.append({
            "qT": np.ascontiguousarray(
                qs.transpose(2, 0, 1).reshape(DK, N * IBLK)),
            "kT": np.ascontiguousarray(
                k[b].transpose(2, 0, 1).reshape(DK, N * LQ)),
            "vS": np.ascontiguousarray(
                v[b].reshape(N, LQ // 128, 128, DV)
                .transpose(2, 0, 1, 3).reshape(128, -1)),
            "d0T": np.ascontiguousarray(d0[b, i0:i0 + IBLK, :].T),
            "d1T": np.ascontiguousarray(d1[b].T),
            "W1d": W1,
            "wexpd": wexp,
            "b1d": b1c,
        })

    res = run_bass_kernel_spmd(nc, in_maps, list(range(NCORES)))
    LAST_RESULTS = res

    output = np.empty((B, N, LQ, DV), np.float32)
    attn = np.empty((B, N, LQ, LQ), np.float32)
    decisions = np.empty((B, 1, LQ, LQ), bool)
    for c in range(NCORES):
        b = c // 4
        i0 = (c % 4) * IBLK
        r = res.results[c]
        output[b, :, i0:i0 + IBLK, :] = r["outp"]
        attn[b, :, i0:i0 + IBLK, :] = r["attn"]
        decisions[b, 0, i0:i0 + IBLK, :] = r["dec"].astype(bool)

    return output, attn, decisions
